# revision 55
# baseline (speedup 1.0000x reference)
"""8-core tensor-parallel multi-head attention (GQA) for TRN2 — v2.

Problem: x[2,2048,1024] -> QKV proj -> 16-head attention (4 KV heads,
GQA groups of 4) -> out proj.  Sharding: 2 query heads + their 1 KV
head per core (tensor parallel); o_proj row-parallel with host-side
partial-sum reduce.

Per-core dataflow (everything transposed so no activation transposes
are needed on the hot path):
  QT[j,n]  = (Wq_i.T x.T):  lhsT=Wq chunk, rhs=xT chunk   (j = 2 heads x 64)
  KVT[j,n] = same with [Wv|Wk] columns (V rows 0:64, K rows 64:128)
  KT2      = K rows duplicated to partitions 0:64 and 64:128 so the two
             heads' S^T matmuls land in disjoint PE row-groups
  S^T[k,q] = KT_h.T @ QT_h          (per 128-row k-tile, 512-col q-tile)
  P^T      = exp(S^T * scale):
               h0: ScalarE ACT Exp (fp32 psum -> bf16)
               h1: VectorE "Schraudolph" exp — one tensor_scalar
                   bits=round(S*scale*A16+B16) -> int16, bitcast bf16.
                   (bf16 bit-space linear interp of 2^x; ~1.8% rms per
                   element, cancels via the shared softmax denominator.)
             Splitting exp across two engines breaks the v1 bottleneck
             (128 ACT exps x 1.11us = 142us on one engine).
  [s;O^T]  = [1|V].T @ P^T          (ones column accumulates the softmax
                                     denominator for free; V rows 0:64)
  OT[j,n]  = O^T * (1/s)            (approx-recip + gpsimd partition
                                     broadcast; sums-row staging on gpsimd)
  out[n,m] = OT.T @ Wo_i            (partial; host sums partials + bo)

v2 perf changes vs v1 (311us -> ~289us, all validated via perfetto/NTFF
trace analysis; clock-state (P0 2.0GHz vs 2.4GHz) varies run-to-run so
wall times carry ~10% noise):
  - exp split ACT/DVE (above) — the kt loop is no longer ACT-bound
    (v1: 128 exps x 1.11us = 142us serial on ScalarE)
  - per-qt softmax-normalize chain is SPREAD over the next q-tile's
    early kts (post_q) instead of emitted inline, and the interleaved
    o_proj fill of the previous tile is gated (oproj_q, delay 5) until
    OT is ready — otherwise the strict-FIFO PE queue stalls on it
  - gpsimd runs a SINGLE op type (one partition_broadcast per q-tile
    on stacked recip rows): mixing broadcast+mul on gpsimd caused
    ~6us Q7 UNLOAD_LIB/LOAD_LIB library swaps on the OT critical path
  - startup ordered KV-proj -> kt2/V-transposes -> Q(0,0), Q(0,1)
    deferred into the loop; x chunks round-robin 3 DMA queues
    (sync/gpsimd/scalar); chunk-major accumulation so the PE starts
    on the first x chunk
  - blocked HBM layouts for x and out (contiguous 256KB chunks);
    tail o_proj writes split across sync+gpsimd queues
"""

import os
import sys

import numpy as np

for _p in ("/opt/trn_rl_repo", "/root/.axon_site/_ro/trn_rl_repo"):
    if os.path.isdir(_p) and _p not in sys.path:
        sys.path.append(_p)

import concourse.bass as bass
import concourse.tile as tile
from concourse import bacc, mybir
from concourse.bass_utils import run_bass_kernel_spmd

AF = mybir.ActivationFunctionType
ALU = mybir.AluOpType
F32 = mybir.dt.float32
I16 = mybir.dt.int16

B, N, D = 2, 2048, 1024
BN = B * N
HEADS, KV_HEADS, HD = 16, 4, 64
SCALE = HD ** -0.5
NCORES = 8
HPC = HEADS // NCORES          # query heads per core = 2
JC = HPC * HD                  # per-core head-dim columns = 128
KC = D // 128                  # contraction chunks for projections = 8
PSD = 512                      # matmul moving free-dim / psum bank size
KTS = N // 128                 # key tiles per batch = 16
QW = 1024                      # attention q-tile width
NQT = N // QW                  # q tiles per batch = 2

# Schraudolph bf16-space exp constants: bits = round(x*A16 + B16)
SCH_A = 2.0 ** 7 / float(np.log(2.0))
SCH_B = 127.0 * 2 ** 7 - 0.0437 * 2 ** 7
# fp8 projection weights are scaled x32 on the host (their 0.02-scale
# values fall into e4m3 subnormals otherwise); undone at bias-add time
PSCALE = 32.0

MM_MODE = os.environ.get("KERNEL_MM_DTYPE", "bfloat16")

_NC_CACHE: dict = {}


def _storage_dt(mode):
    if mode == "bfloat16":
        return mybir.dt.bfloat16
    if mode == "float32r":
        return mybir.dt.float32r
    return F32


def _np_dt(mode):
    if mode == "bfloat16":
        import ml_dtypes
        return ml_dtypes.bfloat16
    return np.float32


def _build_program(mode):
    sdt = _storage_dt(mode)
    filler = int(os.environ.get("KERNEL_FILLER", "8"))
    pump_delay = int(os.environ.get("KERNEL_PUMP_DELAY", "2"))
    use_sch = os.environ.get("KERNEL_SCH", "1") == "1"
    ldw_dedup = os.environ.get("KERNEL_LDW_DEDUP", "1") == "1"
    if sdt == F32 or sdt == mybir.dt.float32r:
        filler = 0  # ldweights rejects fp32/fp32r

    nc = bacc.Bacc("TRN2", target_bir_lowering=False, debug=False)
    use_fp8 = os.environ.get("KERNEL_FP8_PROJ", "0") == "1"
    pdt = mybir.dt.float8e4 if use_fp8 else sdt
    PINV = 1.0 / PSCALE if use_fp8 else 1.0

    # Blocked x: chunk (t, c) = [128, QW] contiguous (t = b*NQT+nq).
    # fp8 x/weights for the projections halves the startup x-DMA bytes
    # (the PE idles ~16us on x-chunk waits); QKV outputs stay bf16.
    xb = nc.dram_tensor("xb", [B * NQT, KC, 128, QW], pdt, kind="ExternalInput")
    # Blocked weights: wq/wkv chunk c = [128, JC] contiguous
    wq = nc.dram_tensor("wq", [KC, 128, JC], pdt, kind="ExternalInput")
    wkv = nc.dram_tensor("wkv", [KC, 128, JC], pdt, kind="ExternalInput")
    wo = nc.dram_tensor("wo", [JC, D], sdt, kind="ExternalInput")
    bq = nc.dram_tensor("bq", [JC, 1], F32, kind="ExternalInput")
    bkv = nc.dram_tensor("bkv", [JC, 1], F32, kind="ExternalInput")
    ident_d = nc.dram_tensor("ident", [64, 64], sdt, kind="ExternalInput")
    ones_d = nc.dram_tensor("ones", [128, KTS], sdt, kind="ExternalInput")
    # Blocked out: chunk (g, mh) = [128, PSD] contiguous, g = global n-tile.
    # bf16 partials: halves the 16.8MB of output DMA (host sums in f64;
    # adds ~4e-4 abs err, well inside the rel-err budget)
    out = nc.dram_tensor("out", [BN // 128, 2, 128, PSD], sdt,
                         kind="ExternalOutput")

    with tile.TileContext(nc) as tc:
        with (
            tc.tile_pool(name="consts", bufs=1) as consts,
            tc.tile_pool(name="xin", bufs=3) as xin,
            tc.tile_pool(name="big", bufs=1) as big,
            tc.tile_pool(name="ptp", bufs=10) as ptp,
            tc.tile_pool(name="stat", bufs=2) as stat,
            tc.tile_pool(name="outp", bufs=6) as outp,
            tc.tile_pool(name="psmm", bufs=2, space="PSUM") as psmm,
            tc.tile_pool(name="psot", bufs=2, space="PSUM") as psot,
        ):
            wq_sb = consts.tile([128, KC, 128], pdt, tag="wq")
            wkv_sb = consts.tile([128, KC, 128], pdt, tag="wkv")
            wo_sb = consts.tile([128, D], sdt, tag="wo")
            bq_sb = consts.tile([128, 1], F32, tag="bq")
            bkv_sb = consts.tile([128, 1], F32, tag="bkv")
            ident = consts.tile([64, 64], sdt, tag="ident")
            # constants on the SWDGE (gpsimd) queue; x streams split over
            # both queues so neither serializes the other at startup
            for c in range(KC):
                nc.gpsimd.dma_start(wq_sb[:, c, :], wq[c])
                nc.gpsimd.dma_start(wkv_sb[:, c, :], wkv[c])
            nc.gpsimd.dma_start(bq_sb[:], bq[:])
            nc.gpsimd.dma_start(bkv_sb[:], bkv[:])
            nc.gpsimd.dma_start(ident[:], ident_d[:])

            QT, KVT, KT2, VO, OT = {}, {}, {}, {}, {}
            for b in range(B):
                QT[b] = big.tile([128, N], sdt, tag=f"QT{b}", name=f"QT{b}")
                KVT[b] = big.tile([128, N], sdt, tag=f"KVT{b}", name=f"KVT{b}")
                KT2[b] = big.tile([128, KTS, 128], sdt, tag=f"KT2{b}",
                                  name=f"KT2{b}")
                VO[b] = big.tile([128, KTS, 65], sdt, tag=f"VO{b}", name=f"VO{b}")
                OT[b] = big.tile([128, N], sdt, tag=f"OT{b}", name=f"OT{b}")
                nc.gpsimd.dma_start(
                    VO[b][:, :, 64:65], ones_d[:].rearrange("p (k o) -> p k o", o=1)
                )

            def dummy_fill(n):
                for _ in range(n):
                    nc.tensor.ldweights(ident[:, 0:1])

            # ---- x loads: one contiguous 256KB descriptor per chunk,
            # alternating queues ----
            xts = {}

            def emit_xt_load(b, nq, four_q=False):
                t = b * NQT + nq
                xt = xin.tile([128, KC, QW], pdt, tag="xt", name=f"xt{b}{nq}")
                engs = ([nc.sync, nc.gpsimd, nc.scalar] if four_q
                        else [nc.sync, nc.gpsimd])
                for c in range(KC):
                    engs[c % len(engs)].dma_start(xt[:, c, :], xb[t, c])
                xts[(b, nq)] = xt

            def emit_kt2(b, half=None):
                kv_blk = KVT[b][64:128, :].rearrange("p (k c) -> p k c", c=128)
                ks = slice(0, KTS) if half is None else (
                    slice(0, KTS // 2) if half == 0 else slice(KTS // 2, KTS))
                nc.sync.dma_start(KT2[b][0:64, ks, :], kv_blk[:, ks, :])
                nc.sync.dma_start(KT2[b][64:128, ks, :], kv_blk[:, ks, :])

            def emit_transpose_pair(b, kt0):
                for kt in (kt0, kt0 + 1):
                    vps = psmm.tile([128, 64], sdt, tag="mm")
                    nc.tensor.transpose(
                        vps[:], KVT[b][0:64, kt * 128 : (kt + 1) * 128], ident[:]
                    )
                    if kt % 2 == 0:
                        nc.vector.tensor_copy(VO[b][:, kt, 0:64], vps[:])
                    else:
                        nc.scalar.copy(VO[b][:, kt, 0:64], vps[:])

            # ---- projections ----
            def emit_proj_chunkmajor(b, nq, which):
                """Chunk-major projection of one weight set for one n-tile:
                per x-chunk c, 2 matmuls (2 halves) accumulate into 2 psum
                tiles, so the PE starts as soon as chunk 0 lands."""
                ns = nq * QW
                xt = xts[(b, nq)]
                wsb = wq_sb if which == 0 else wkv_sb
                ps = [
                    psmm.tile([128, PSD], F32, tag="mm", name=f"pj{b}{nq}{which}{h}")
                    for h in range(2)
                ]
                for c in range(KC):
                    for half in range(2):
                        sl = slice(half * PSD, (half + 1) * PSD)
                        nc.tensor.matmul(
                            ps[half][:], wsb[:, c, :], xt[:, c, sl],
                            start=(c == 0), stop=(c == KC - 1),
                        )
                dst, bias = (QT[b], bq_sb) if which == 0 else (KVT[b], bkv_sb)
                for half in range(2):
                    sl = slice(ns + half * PSD, ns + (half + 1) * PSD)
                    if which == 0:
                        nc.vector.tensor_scalar(
                            dst[:, sl], ps[half][:], PINV, bias[:],
                            op0=ALU.mult, op1=ALU.add)
                    else:
                        nc.scalar.activation(
                            dst[:, sl], ps[half][:], AF.Identity,
                            bias=bias[:], scale=PINV)

            def emit_proj_chunk(b, nq, which, half):
                """n-major variant used for the in-loop (b=1) fill work: 8
                accumulating matmuls + bias for one 512-wide half."""
                ns = nq * QW
                wsb, dst, bias = (
                    (wq_sb, QT[b], bq_sb) if which == 0 else (wkv_sb, KVT[b], bkv_sb)
                )
                xt = xts[(b, nq)]
                sl = slice(half * PSD, (half + 1) * PSD)
                ps = psmm.tile([128, PSD], F32, tag="mm")
                for c in range(KC):
                    nc.tensor.matmul(
                        ps[:], wsb[:, c, :], xt[:, c, sl],
                        start=(c == 0), stop=(c == KC - 1),
                    )
                if which == 0:
                    nc.vector.tensor_scalar(
                        dst[:, ns + half * PSD : ns + (half + 1) * PSD],
                        ps[:], PINV, bias[:], op0=ALU.mult, op1=ALU.add)
                else:
                    nc.scalar.activation(
                        dst[:, ns + half * PSD : ns + (half + 1) * PSD],
                        ps[:], AF.Identity, bias=bias[:], scale=PINV)

            # startup, ordered for earliest attention start: KV proj first
            # (attention needs ALL keys/values), kt2-dup + V-transposes per
            # half as KVT completes, then Q of the first n-tile only;
            # Q(0,1) is deferred into the loop as fill work.
            for nq in range(NQT):
                emit_xt_load(0, nq, four_q=True)
            nc.gpsimd.dma_start(wo_sb[:], wo[:])
            emit_proj_chunkmajor(0, 0, 1)
            emit_kt2(0, half=0)
            for kt0 in range(0, KTS // 2, 2):
                emit_transpose_pair(0, kt0)
            emit_proj_chunkmajor(0, 1, 1)
            emit_kt2(0, half=1)
            emit_proj_chunkmajor(0, 0, 0)
            for kt0 in range(KTS // 2, KTS, 2):
                emit_transpose_pair(0, kt0)
            for nq in range(NQT):
                emit_xt_load(1, nq)

            # ---- o_proj of a finished q-tile (interleaved as fill work) ----
            def emit_oproj_chunk(b, qs, nt, mh, pool=None, tail=False):
                ns = qs + nt * 128
                g = (b * N + ns) // 128
                ops = (pool or psmm).tile([128, PSD], F32,
                                          tag="mm" if pool is None else "ot")
                nc.tensor.matmul(
                    ops[:], OT[b][:, ns : ns + 128],
                    wo_sb[:, mh * PSD : (mh + 1) * PSD],
                )
                osb = outp.tile([128, PSD], sdt, tag="osb")
                if (nt * 2 + mh) % 2 == 0:
                    nc.vector.tensor_copy(osb[:], ops[:])
                else:
                    nc.scalar.copy(osb[:], ops[:])
                # in the tail the sync queue serializes ~22 chunk writes;
                # spread across both DMA queues there
                eng = nc.gpsimd if (tail and nt % 2 == 1) else nc.sync
                eng.dma_start(out[g, mh], osb[:])

            prev = None  # (b, qs) whose o_proj still needs emitting
            from collections import deque
            fillq = deque()       # dependency-light PE fill (proj, transposes)
            oproj_q = deque()     # (qt_idx, thunk): o_proj, gated on OT ready
            post_q = deque()      # deferred normalize ops of the PREVIOUS qt,
                                  # spread over this qt's early kts so the
                                  # ACT/DVE FIFOs aren't blocked at boundaries
            oproj_delay = int(os.environ.get("KERNEL_OPROJ_DELAY", "5"))

            def pump(kt, cur_idx):
                if fillq:
                    fillq.popleft()()
                elif oproj_q and (
                    oproj_q[0][0] < cur_idx - 1 or kt >= oproj_delay
                ):
                    oproj_q.popleft()[1]()
                elif filler:
                    dummy_fill(filler)

            def emit_normalize(b, qs, osbs, spread):
                """Per-head softmax normalization of the evacuated AV
                accumulators: stage sums row (partition 64 -> 0, ACT — the
                one engine proven to shift partitions), r=1/sums, broadcast,
                scale O^T into OT.  spread=True queues the steps for per-kt
                emission inside the next q-tile's loop so the engine FIFOs
                aren't blocked at the boundary."""
                # both heads' recip rows stacked in one tile so gpsimd does a
                # SINGLE partition_broadcast per q-tile: gpsimd then runs only
                # one op type and never swaps its Q7 software library (an
                # UNLOAD_LIB/LOAD_LIB pair costs ~6us and sat on the OT
                # critical path)
                r2 = stat.tile([1, 2 * QW], F32, tag="r2")
                rb2 = stat.tile([64, 2 * QW], F32, tag="rb2")
                steps = []
                for h in range(2):
                    osb = osbs[h]
                    ssb = stat.tile([1, QW], F32, tag=f"ssb{h}")
                    hs = slice(h * QW, (h + 1) * QW)
                    steps.append(
                        lambda osb=osb, ssb=ssb: nc.scalar.copy(
                            ssb[:], osb[64:65, :])
                    )
                    steps.append(
                        lambda ssb=ssb, hs=hs: nc.vector.reciprocal_approx_fast(
                            r2[:, hs], ssb[:])
                    )
                    # per-head broadcast (still the only gpsimd op type, no
                    # lib swap) so head 0's chain doesn't wait on head 1's
                    # recip — matters for the final tile's inline chain
                    steps.append(
                        lambda hs=hs: nc.gpsimd.partition_broadcast(
                            rb2[:, hs], r2[0:1, hs])
                    )
                    if h == 0:
                        steps.append(
                            lambda osb=osb, hs=hs: nc.vector.tensor_mul(
                                OT[b][0:64, qs : qs + QW], osb[0:64, :],
                                rb2[:, hs])
                        )
                    else:
                        def h1_mul(osb=osb, hs=hs):
                            tmp = stat.tile([64, QW], sdt, tag="tmp")
                            nc.vector.tensor_mul(
                                tmp[:], osb[0:64, :], rb2[:, hs])
                            nc.sync.dma_start(
                                OT[b][64:128, qs : qs + QW], tmp[:])
                        steps.append(h1_mul)
                if spread:
                    post_q.extend(steps)
                else:
                    for s in steps:
                        s()

            for b in range(B):
                for qt in range(NQT):
                    qs = qt * QW
                    cur_idx = b * NQT + qt
                    if b == 0 and qt == 0:
                        for half in range(2):
                            fillq.append(
                                lambda h=half: emit_proj_chunk(0, 1, 0, h))
                        for nq in range(NQT):
                            for which in range(2):
                                for half in range(2):
                                    fillq.append(
                                        (lambda nq=nq, w=which, h=half:
                                         emit_proj_chunk(1, nq, w, h))
                                    )
                    elif b == 0 and qt == 1:
                        emit_kt2(1)
                        for kt0 in range(0, KTS, 2):
                            fillq.append(lambda kt0=kt0: emit_transpose_pair(1, kt0))
                    if prev is not None:
                        pb_, pq_ = prev
                        for nt in range(QW // 128):
                            for mh in range(2):
                                oproj_q.append(
                                    (cur_idx - 1,
                                     lambda nt=nt, mh=mh, pb=pb_, pq=pq_:
                                     emit_oproj_chunk(pb, pq, nt, mh))
                                )
                    o_ps = [
                        psot.tile([65, QW], F32, tag="ot", name=f"ops{h}")
                        for h in range(2)
                    ]
                    pend = None  # pts of previous kt awaiting AV
                    for kt in range(KTS):
                        pts = {}
                        for h in range(2):
                            st = psmm.tile([128, QW], F32, tag="mm")
                            for h2 in range(QW // PSD):
                                sl = slice(h2 * PSD, (h2 + 1) * PSD)
                                bi = nc.tensor.matmul(
                                    st[:, sl],
                                    KT2[b][64 * h : 64 * h + 64, kt, :],
                                    QT[b][64 * h : 64 * h + 64,
                                          qs + h2 * PSD : qs + (h2 + 1) * PSD],
                                )
                                # second h2 chunk reuses the loaded KT2
                                # stationary (sim-validated: the scheduler
                                # keeps the pair adjacent)
                                if ldw_dedup and h2 > 0:
                                    bi.ins.ldweights = False
                            if use_sch and h == 1:
                                pt16 = ptp.tile([128, QW], I16, tag="pt")
                                nc.vector.tensor_scalar(
                                    pt16[:], st[:], SCH_A * SCALE, SCH_B,
                                    op0=ALU.mult, op1=ALU.add,
                                )
                                pv = pt16[:].bitcast(mybir.dt.bfloat16)
                                for h2 in range(QW // PSD):
                                    pts[(h, h2)] = pv[
                                        :, h2 * PSD : (h2 + 1) * PSD]
                            else:
                                pt = ptp.tile([128, QW], sdt, tag="pt")
                                nc.scalar.activation(
                                    pt[:], st[:], AF.Exp, scale=SCALE
                                )
                                for h2 in range(QW // PSD):
                                    pts[(h, h2)] = pt[
                                        :, h2 * PSD : (h2 + 1) * PSD]
                        if pend is not None:
                            pkt, ppts = pend
                            for h in range(2):
                                for h2 in range(QW // PSD):
                                    sl = slice(h2 * PSD, (h2 + 1) * PSD)
                                    bi = nc.tensor.matmul(
                                        o_ps[h][:, sl], VO[b][:, pkt, :],
                                        ppts[(h, h2)],
                                        start=(pkt == 0), stop=(pkt == KTS - 1),
                                    )
                                    # all 4 AV matmuls share the VO stationary
                                    if ldw_dedup and (h or h2):
                                        bi.ins.ldweights = False
                        for _ in range(2):
                            if post_q and kt >= 1:
                                post_q.popleft()()
                        if kt < pump_delay:
                            if filler:
                                dummy_fill(2 * filler)
                        else:
                            pump(kt, cur_idx)
                        pend = (kt, pts)
                    # flush AV of the final kt
                    pkt, ppts = pend
                    for h in range(2):
                        for h2 in range(QW // PSD):
                            sl = slice(h2 * PSD, (h2 + 1) * PSD)
                            bi = nc.tensor.matmul(
                                o_ps[h][:, sl], VO[b][:, pkt, :], ppts[(h, h2)],
                                start=(pkt == 0), stop=(pkt == KTS - 1),
                            )
                            if ldw_dedup and (h or h2):
                                bi.ins.ldweights = False
                    # evacuate both accumulators now (DVE + ACT in parallel:
                    # the next tile's AV needs the psot slots); normalize is
                    # spread into the next q-tile's early kts
                    osbs = []
                    for h in range(2):
                        osb = stat.tile([65, QW], F32, tag=f"osb{h}",
                                        name=f"osb{h}")
                        if h == 0:
                            nc.vector.tensor_copy(osb[:], o_ps[h][:])
                        else:
                            nc.scalar.copy(osb[:], o_ps[h][:])
                        osbs.append(osb)
                    last = (b == B - 1 and qt == NQT - 1)
                    emit_normalize(b, qs, osbs, spread=not last)
                    prev = (b, qs)

            if filler:
                dummy_fill(6 * filler)
            while fillq:
                fillq.popleft()()
            while oproj_q:
                oproj_q.popleft()[1]()

            # o_proj for the final q-tile
            tb, tqs = prev
            for nt in range(QW // 128):
                for mh in range(2):
                    emit_oproj_chunk(tb, tqs, nt, mh,
                                     pool=psot if (nt * 2 + mh) % 2 else None,
                                     tail=True)

    nc.compile()
    return nc


def _get_nc(mode):
    key = (mode,
           os.environ.get("KERNEL_PUMP_DELAY", "2"),
           os.environ.get("KERNEL_FILLER", "8"),
           os.environ.get("KERNEL_OPROJ_DELAY", "5"),
           os.environ.get("KERNEL_LDW_DEDUP", "1"),
           os.environ.get("KERNEL_FP8_PROJ", "0"),
           os.environ.get("KERNEL_SCH", "1"))
    if key not in _NC_CACHE:
        _NC_CACHE[key] = _build_program(mode)
    return _NC_CACHE[key]


def _prep_in_maps(inputs, mode):
    ndt = _np_dt(mode)
    if os.environ.get("KERNEL_FP8_PROJ", "0") == "1":
        import ml_dtypes
        pdt_np = ml_dtypes.float8_e4m3fn
    else:
        pdt_np = ndt
    x = np.asarray(inputs["x"], np.float32)
    Wq = np.asarray(inputs["Wq"], np.float32)
    bq = np.asarray(inputs["bq"], np.float32)
    Wk = np.asarray(inputs["Wk"], np.float32)
    bk = np.asarray(inputs["bk"], np.float32)
    Wv = np.asarray(inputs["Wv"], np.float32)
    bv = np.asarray(inputs["bv"], np.float32)
    Wo = np.asarray(inputs["Wo"], np.float32)

    # blocked x: [B*NQT, KC, 128, QW];  xb[t, c, p, q] = x.T[c*128+p, t*QW+q]
    xT = x.reshape(BN, D).T.astype(pdt_np)           # [D, BN]
    xb = np.ascontiguousarray(
        xT.reshape(KC, 128, B * NQT, QW).transpose(2, 0, 1, 3)
    )
    in_maps = []
    for i in range(NCORES):
        j0 = i * JC              # query-head column offset (heads 2i, 2i+1)
        g = i // 2               # kv head for this core
        v0 = g * HD
        wsc = PSCALE if pdt_np is not ndt else 1.0
        wq_i = np.ascontiguousarray(
            Wq[:, j0 : j0 + JC].reshape(KC, 128, JC) * wsc).astype(pdt_np)
        wkv_i = np.concatenate(
            [Wv[:, v0 : v0 + HD], Wk[:, v0 : v0 + HD]], axis=1
        )  # V cols first (rows 0:64 of KVT), K cols second (rows 64:128)
        wkv_i = np.ascontiguousarray(
            wkv_i.reshape(KC, 128, JC) * wsc).astype(pdt_np)
        bkv_i = np.concatenate([bv[v0 : v0 + HD], bk[v0 : v0 + HD]])
        in_maps.append({
            "xb": xb,
            "wq": wq_i,
            "wkv": wkv_i,
            "wo": np.ascontiguousarray(Wo[j0 : j0 + JC, :]).astype(ndt),
            "bq": np.ascontiguousarray(bq[j0 : j0 + JC]).reshape(JC, 1)
                    .astype(np.float32),
            "bkv": np.ascontiguousarray(bkv_i).reshape(JC, 1).astype(np.float32),
            "ident": np.eye(64, dtype=np.float32).astype(ndt),
            "ones": np.ones((128, KTS), dtype=np.float32).astype(ndt),
        })
    return in_maps


def _run(inputs, trace=False):
    mode = MM_MODE
    nc = _get_nc(mode)
    in_maps = _prep_in_maps(inputs, mode)
    res = run_bass_kernel_spmd(
        nc, in_maps, core_ids=list(range(NCORES)), trace=trace
    )
    bo = np.asarray(inputs["bo"], np.float32)
    acc = res.results[0]["out"].astype(np.float64)
    for i in range(1, NCORES):
        acc += res.results[i]["out"].astype(np.float64)
    # unblock: [32, 2, 128, 512] -> [4096, 1024]
    full = acc.transpose(0, 2, 1, 3).reshape(BN, D)
    full = (full + bo.astype(np.float64)).astype(np.float32).reshape(B, N, D)
    return full, res


def kernel(**inputs):
    return _run(inputs, trace=False)[0]
